# revision 14
# baseline (speedup 1.0000x reference)
"""Trainium2 Bass kernel for causal self-attention with GQA + RoPE.

Model: B=2, T=2048, C=2048, H=16 query heads, H_KV=4 kv heads, D=128.

Sharding (8 NeuronCores, pure SPMD, no collectives):
  core i -> batch b = i // 4, kv-group g = i % 4
            (query heads 4g..4g+3, kv head g, all T positions of batch b).
  Every core runs an identical program; only input data differs.
  o_proj is computed against the row-slice wo[512g:512(g+1), :], giving a
  partial [T, C] output per core; the sum over the 4 cores of each batch
  (the tensor-parallel all-reduce) is done on the host in numpy.

v3 (on top of the bf16 v2 baseline):
  - rowsum pair-reduce: adjacent 128-wide k-subtiles of exp(S) are summed
    pairwise on DVE (bf16 tensor_tensor, 2x mode); the ones-matmul rowsum
    then streams half the columns (PE 29us -> 16us). Cross-pair
    accumulation stays in PSUM fp32 (the single bf16 pair-add rounds
    independently per element -> rowsum error ~2^-9/sqrt(1024), trivial).
    The final (diagonal) group of each head keeps direct rowsum matmuls
    so no cross-head deferral is needed.
  - o_proj PSUM evacuation alternates vector/scalar (was all-vector, which
    serialized the o_unit pipeline through one engine).
  - tail restructure: the attention-only PSUM pools close before the last
    chunk's o_proj, freeing 6 banks; the tail runs from a 4-deep PSUM pool
    with stores on sync/scalar. gpsimd (slow ~7.6us software-DGE drain)
    issues no DMA after mid-attention.

v4:
  - big-DMA startup: one dma_start is split across all 16 SDMA engines of
    its queue, so large transfers run ~170GB/s vs ~90GB/s for a stream of
    128KB descriptors. Phase-0 loads a few leading singles (so the PE can
    start at ~9.5us, right after the ~7.2us framework preamble) and then
    grouped 0.6-1MB transfers. wk|wv are concatenated host-side into one
    wkv tensor (one descriptor per cc on gpsimd).
  - x for q-chunks 1..3 prefetched as 2x1MB transfers (sync/scalar), one
    chunk ahead.
  - rope restructure: the six PSUM evacuations of a chunk's projections
    are emitted back-to-back at chunk end, split scalar/vector; the rope
    rotate-matmuls + V transposes of chunk qc are interleaved into chunk
    qc+1's projection stream (the PE never waits on the evac/TT chain);
    the rotate/transpose PSUM tag is double-buffered. This removes the
    ~2-3us PE stall at every chunk boundary that also re-throttled the
    PE clock (HAM) mid-kernel.
"""

import math
import os

import numpy as np

os.environ.setdefault("MYCRO_LOCAL_CACHE", "1")

P = 128
D = 128
H = 16
H_KV = 4
GQ = H // H_KV  # 4 query heads per kv head (= per core)
B = 2
T_FULL = 2048
C_DIM = 2048
NCORES = 8
ROPE_BASE = 10000.0


def _rope_tables(T):
    inv_freq = 1.0 / (ROPE_BASE ** (np.arange(0, D, 2, dtype=np.float32) / D))
    t = np.arange(T, dtype=np.float32)
    freqs = np.outer(t, inv_freq)  # [T, D/2]
    emb = np.concatenate((freqs, freqs), axis=-1)  # [T, D]
    return (
        np.ascontiguousarray(np.cos(emb).T.astype(np.float32)),  # [D, T]
        np.ascontiguousarray(np.sin(emb).T.astype(np.float32)),
    )


def _rot_lhsT():
    # rotate_half(q) = R @ q with R[d, d+64] = -1 (d < 64), R[d, d-64] = +1.
    # matmul computes lhsT.T @ rhs, so pass lhsT = R^T.
    R = np.zeros((D, D), dtype=np.float32)
    half = D // 2
    R[np.arange(half), np.arange(half) + half] = -1.0
    R[np.arange(half) + half, np.arange(half)] = 1.0
    return np.ascontiguousarray(R.T)


def _tri128():
    # tri[k, j] = 1 if j >= k else 0: the in-subtile causal triangle after
    # diagonal narrowing (column j of a narrowed diag slice is q = 128m + j,
    # row k is k_local; valid iff j >= k).
    k = np.arange(P)
    return (k[None, :] >= k[:, None]).astype(np.float32)


def build_nc(T=T_FULL):
    """Build the per-core Bass/Tile program (identical across cores)."""
    from contextlib import ExitStack

    import concourse.mybir as mybir
    import concourse.tile as tile
    from concourse import bacc
    from concourse.masks import make_identity

    f32 = mybir.dt.float32
    bf16 = mybir.dt.bfloat16
    Exp = mybir.ActivationFunctionType.Exp
    MULT = mybir.AluOpType.mult
    ADD = mybir.AluOpType.add
    SCALE = 1.0 / math.sqrt(D)

    NCC = C_DIM // P  # 16 contraction chunks
    NQC = T // 512  # projection / attention q-chunks (512-wide)
    NCT = C_DIM // 512  # o_proj column tiles
    NKB = T // P  # 128-wide k subtiles

    nc = bacc.Bacc(
        "TRN2",
        target_bir_lowering=False,
        debug=False,
        num_devices=NCORES,
    )

    xt = nc.dram_tensor("xt", [C_DIM, T], bf16, kind="ExternalInput").ap()
    wq = nc.dram_tensor("wq", [C_DIM, GQ * D], bf16, kind="ExternalInput").ap()
    wkv = nc.dram_tensor("wkv", [C_DIM, 2 * D], bf16, kind="ExternalInput").ap()
    wo = nc.dram_tensor("wo", [GQ * D, C_DIM], bf16, kind="ExternalInput").ap()
    cosT = nc.dram_tensor("cosT", [D, T], bf16, kind="ExternalInput").ap()
    sinT = nc.dram_tensor("sinT", [D, T], bf16, kind="ExternalInput").ap()
    trim = nc.dram_tensor("trim", [P, P], bf16, kind="ExternalInput").ap()
    onesm = nc.dram_tensor("onesm", [P, P], bf16, kind="ExternalInput").ap()
    rotm = nc.dram_tensor("rotm", [P, P], bf16, kind="ExternalInput").ap()
    out = nc.dram_tensor("out", [T, C_DIM], bf16, kind="ExternalOutput").ap()

    with tile.TileContext(nc) as tc, ExitStack() as ctx:
        const = ctx.enter_context(tc.tile_pool(name="const", bufs=1))
        acts = ctx.enter_context(tc.tile_pool(name="acts", bufs=1))

        wq_r = wq.rearrange("(cc p) n -> p cc n", p=P)
        wkv_r = wkv.rearrange("(cc p) n -> p cc n", p=P)
        xt_r = xt.rearrange("(cc p) t -> p cc t", p=P)
        wo_r = wo.rearrange("(h p) (ct n) -> p h ct n", p=P, n=512)

        ones_sb = const.tile([P, P], bf16)
        rot_sb = const.tile([P, P], bf16)
        ident = const.tile([P, P], bf16)
        tri_sb = const.tile([P, P], bf16)

        # long-lived activations (all bf16: 44KB/partition total)
        qt_sb = [acts.tile([P, T], bf16, name=f"qt{h}") for h in range(GQ)]
        kt_sb = acts.tile([P, T], bf16, name="kt")
        v_sb = acts.tile([P, NKB, D], bf16, name="vnat")
        y_sb = [acts.tile([P, T], bf16, name=f"yt{h}") for h in range(GQ)]
        wo_sb = acts.tile([P, GQ, NCT, 512], bf16, name="wo_sb")

        # ---------------- phase 1: projections + rope ----------------
        with (
            tc.tile_pool(name="pwts", bufs=1) as wpool,
            tc.tile_pool(name="xts", bufs=4) as xt_pool,
            tc.tile_pool(name="rope_t", bufs=1) as rope_pool,
            tc.tile_pool(name="proj_ps", bufs=1, space="PSUM") as proj_ps,
            tc.tile_pool(name="aux_ps", bufs=1, space="PSUM") as aux_ps,
            tc.tile_pool(name="ptmp", bufs=2) as ptmp,
        ):
            wq_sb = wpool.tile([P, NCC, GQ * D], bf16)
            wkv_sb = wpool.tile([P, NCC, 2 * D], bf16)
            lead_xs = xt_pool.tile([P, NCC, 512], bf16, tag="xlead",
                               name="lead_xs", bufs=1)
            cos_sb = rope_pool.tile([P, T], bf16)
            sin_sb = rope_pool.tile([P, T], bf16)

            # identity first: two cheap gpsimd ops, then gpsimd is free to
            # issue DMA descriptors.
            make_identity(nc, ident)

            # Phase-0 loads. One dma_start is striped across all 16 SDMA
            # engines of its queue: big transfers run ~170GB/s while 128KB
            # singles only reach ~90GB/s. Lead with singles so the first
            # matmuls can start right after the ~7.2us framework preamble,
            # then switch to grouped transfers that outrun the PE.
            for cc in range(4):
                nc.sync.dma_start(lead_xs[:, cc, :], xt_r[:, cc, 0:512])
                nc.scalar.dma_start(wq_sb[:, cc, :], wq_r[:, cc, :])
            nc.sync.dma_start(lead_xs[:, 4:10, :], xt_r[:, 4:10, 0:512])
            nc.sync.dma_start(lead_xs[:, 10:16, :], xt_r[:, 10:16, 0:512])
            nc.scalar.dma_start(wq_sb[:, 4:10, :], wq_r[:, 4:10, :])
            nc.scalar.dma_start(wq_sb[:, 10:16, :], wq_r[:, 10:16, :])
            for cc in range(6):
                nc.gpsimd.dma_start(wkv_sb[:, cc, :], wkv_r[:, cc, :])
            nc.gpsimd.dma_start(wkv_sb[:, 6:16, :], wkv_r[:, 6:16, :])
            # rope tables / consts on gpsimd (rotm + chunk-0 tables needed
            # from ~33us; the rest later).
            nc.gpsimd.dma_start(rot_sb[:], rotm)
            nc.gpsimd.dma_start(cos_sb[:, 0:512], cosT[:, 0:512])
            nc.gpsimd.dma_start(sin_sb[:, 0:512], sinT[:, 0:512])
            nc.gpsimd.dma_start(tri_sb[:], trim)
            nc.gpsimd.dma_start(ones_sb[:], onesm)
            nc.gpsimd.dma_start(cos_sb[:, 512:T], cosT[:, 512:T])
            nc.gpsimd.dma_start(sin_sb[:, 512:T], sinT[:, 512:T])
            # wo preload (needed ~115us) as two 1MB transfers.
            nc.gpsimd.dma_start(wo_sb[:, 0:2, :, :], wo_r[:, 0:2, :, :])
            nc.gpsimd.dma_start(wo_sb[:, 2:4, :, :], wo_r[:, 2:4, :, :])
            # warm the ACT exp table set during the initial DMA wait
            warm = ptmp.tile([P, 1], f32, name="warm", tag="warm")
            nc.scalar.activation(warm[:], warm[:], Exp)

            def rot_tt(raw, dst, cosq, sinq):
                # dst = raw*cos + (R raw)*sin
                rp = aux_ps.tile([P, 512], f32, name="rotp", tag="rotp",
                                 bufs=2)
                nc.tensor.matmul(rp[:], rot_sb[:], raw[:], start=True,
                                 stop=True)
                nc.vector.tensor_tensor(dst, raw[:], cosq, MULT)
                t2 = ptmp.tile([P, 512], bf16, name="rt2", tag="rt2")
                nc.vector.tensor_tensor(t2[:], rp[:], sinq, MULT)
                nc.vector.tensor_tensor(dst, dst, t2[:], ADD)

            # pending rope work of the previous chunk, interleaved into the
            # current chunk's projection stream so the PE never waits on
            # the PSUM-evacuation/TT chain.
            pend = None  # (qc_prev, raws[4], rawk, vraw)
            xh_next = None

            for qc in range(NQC):
                q0 = qc * 512
                if qc == 0:
                    xt_all = lead_xs
                else:
                    xt_all = xh_next
                # prefetch next chunk's x as two 1MB transfers
                if qc + 1 < NQC:
                    nq0 = (qc + 1) * 512
                    xh = xt_pool.tile([P, NCC, 512], bf16, tag="xh",
                                      name="xh", bufs=2)
                    nc.sync.dma_start(xh[:, 0:8, :],
                                      xt_r[:, 0:8, nq0 : nq0 + 512])
                    nc.scalar.dma_start(xh[:, 8:16, :],
                                        xt_r[:, 8:16, nq0 : nq0 + 512])
                    xh_next = xh

                qp = [
                    proj_ps.tile([P, 512], f32, name=f"qp{h}", tag=f"qp{h}")
                    for h in range(GQ)
                ]
                kp = proj_ps.tile([P, 512], f32, name="kp", tag="kp")
                vp = proj_ps.tile([P, 512], f32, name="vp", tag="vp")
                for cc in range(NCC):
                    xtile = xt_all[:, cc, :]
                    first, last = cc == 0, cc == NCC - 1
                    for h in range(GQ):
                        nc.tensor.matmul(
                            qp[h][:],
                            wq_sb[:, cc, h * D : (h + 1) * D],
                            xtile,
                            start=first,
                            stop=last,
                        )
                    nc.tensor.matmul(
                        kp[:], wkv_sb[:, cc, 0:D], xtile, start=first,
                        stop=last
                    )
                    nc.tensor.matmul(
                        vp[:], wkv_sb[:, cc, D : 2 * D], xtile, start=first,
                        stop=last
                    )
                    # previous chunk's rope/V-transpose work as filler
                    if pend is not None and cc in (0, 1, 2, 4, 5):
                        pq, raws, rawk, vraw = pend
                        pq0 = pq * 512
                        pcos = cos_sb[:, pq0 : pq0 + 512]
                        psin = sin_sb[:, pq0 : pq0 + 512]
                        if cc == 0:
                            rot_tt(raws[0], qt_sb[0][:, pq0 : pq0 + 512],
                                   pcos, psin)
                            rot_tt(raws[1], qt_sb[1][:, pq0 : pq0 + 512],
                                   pcos, psin)
                        elif cc == 1:
                            rot_tt(raws[2], qt_sb[2][:, pq0 : pq0 + 512],
                                   pcos, psin)
                            rot_tt(raws[3], qt_sb[3][:, pq0 : pq0 + 512],
                                   pcos, psin)
                        elif cc == 2:
                            rot_tt(rawk, kt_sb[:, pq0 : pq0 + 512],
                                   pcos, psin)
                        elif cc in (4, 5):
                            for ks in ((0, 1) if cc == 4 else (2, 3)):
                                tp = aux_ps.tile([P, P], bf16, name="vtrp",
                                                 tag="rotp", bufs=2)
                                nc.tensor.transpose(
                                    tp[:], vraw[:, ks * P : (ks + 1) * P],
                                    ident[:],
                                )
                                nc.vector.tensor_copy(
                                    v_sb[:, pq * 4 + ks, :], tp[:]
                                )

                # end of chunk: evacuate all six projection accumulators,
                # split across scalar and vector so the wave is ~2x faster;
                # the rope matmuls run inside the next chunk's projections.
                raws = []
                for h in range(GQ):
                    raw = ptmp.tile([P, 512], bf16, name=f"rraw{h}",
                                    tag=f"rraw{h}", bufs=1)
                    if h % 2 == 0:
                        nc.scalar.copy(raw[:], qp[h][:])
                    else:
                        nc.vector.tensor_copy(raw[:], qp[h][:])
                    raws.append(raw)
                rawk = ptmp.tile([P, 512], bf16, name="rawk", tag="rawk",
                                 bufs=1)
                nc.scalar.copy(rawk[:], kp[:])
                vraw = ptmp.tile([P, 512], bf16, name="vraw", tag="vraw",
                                 bufs=1)
                nc.vector.tensor_copy(vraw[:], vp[:])
                pend = (qc, raws, rawk, vraw)

            # rope + V-transpose for the last chunk (no projection stream
            # left to hide it; rotp is double-buffered so it pipelines).
            pq, raws, rawk, vraw = pend
            pq0 = pq * 512
            pcos = cos_sb[:, pq0 : pq0 + 512]
            psin = sin_sb[:, pq0 : pq0 + 512]
            for h in range(GQ):
                rot_tt(raws[h], qt_sb[h][:, pq0 : pq0 + 512], pcos, psin)
            rot_tt(rawk, kt_sb[:, pq0 : pq0 + 512], pcos, psin)
            for ks in range(4):
                tp = aux_ps.tile([P, P], bf16, name="vtrp", tag="rotp",
                                 bufs=2)
                nc.tensor.transpose(tp[:], vraw[:, ks * P : (ks + 1) * P],
                                    ident[:])
                nc.vector.tensor_copy(v_sb[:, pq * 4 + ks, :], tp[:])

        # -------- phase 2: causal attention + interleaved o_proj --------
        with (
            tc.tile_pool(name="pt_pool", bufs=3) as pt_pool,
            tc.tile_pool(name="o_ps", bufs=2, space="PSUM") as o_ps,
            tc.tile_pool(name="nrm", bufs=2) as nrm_pool,
            tc.tile_pool(name="ost", bufs=4) as ost_pool,
            tc.tile_pool(name="pairs", bufs=3) as pair_pool,
        ):
            o_count = [0]
            o_queues = (nc.sync, nc.scalar, nc.gpsimd)
            evac_engines = (nc.vector, nc.scalar)

            def o_unit(aq, ct, qb, ps_pool, store_queues):
                # one o_proj output tile [128 q rows, 512 cols] for chunk aq
                op = ps_pool.tile([P, 512], f32, name="op", tag="op")
                for h in range(GQ):
                    nc.tensor.matmul(
                        op[:],
                        y_sb[h][:, qb * P : (qb + 1) * P],
                        wo_sb[:, h, ct, :],
                        start=(h == 0),
                        stop=(h == GQ - 1),
                    )
                ot = ost_pool.tile([P, 512], bf16, name="ot", tag="ot")
                ev = evac_engines[o_count[0] % 2]
                if ev is nc.scalar:
                    nc.scalar.copy(ot[:], op[:])
                else:
                    nc.vector.tensor_copy(ot[:], op[:])
                oq = store_queues[o_count[0] % len(store_queues)]
                o_count[0] += 1
                oq.dma_start(
                    out[qb * P : (qb + 1) * P, ct * 512 : (ct + 1) * 512],
                    ot[:],
                )

            def make_units(aq):
                return [(aq, ct, qb) for ct in range(NCT)
                        for qb in range(4 * aq, 4 * aq + 4)]

            with (
                tc.tile_pool(name="s_ps", bufs=2, space="PSUM") as s_ps,
                tc.tile_pool(name="y_ps", bufs=1, space="PSUM") as y_ps,
                tc.tile_pool(name="rs_ps", bufs=1, space="PSUM") as rs_ps,
            ):
                for aq in range(NQC):
                    q0 = aq * 512
                    nks = 4 * (aq + 1)  # 128-wide k subtiles (incl 4 diagonal)
                    ng = nks // 2  # groups of 2 subtiles
                    units = make_units(aq - 1) if aq > 0 else []
                    slots = GQ * ng
                    credit = 0.0
                    ucount = len(units)

                    # narrowed (offset, width) per k-subtile: diagonal subtile
                    # m only covers q >= 128m within the 512-wide chunk.
                    def ow(ks):
                        m = ks - (nks - 4)
                        if m > 0:
                            return 128 * m, 512 - 128 * m
                        return 0, 512

                    for h in range(GQ):
                        qrow = qt_sb[h]
                        yp = y_ps.tile([P, 512], f32, name="yp", tag="yp")
                        rp_ = rs_ps.tile([P, 512], f32, name="rsp", tag="rsp")
                        sps = [None] * ng
                        # pair tiles awaiting their rowsum matmul:
                        # list of (tile, offA) in group order
                        pend_pairs = [None] * ng

                        def s_issue(g):
                            # the two subtiles are packed back to back in the
                            # sp tile ([0:w0], [w0:w0+w1]); w0 is always 256
                            # or 512 so neither matmul output crosses a PSUM
                            # bank.
                            sp = s_ps.tile([P, 1024], f32, name="sp", tag="sp")
                            off1 = 0
                            for ks in (2 * g, 2 * g + 1):
                                off, w = ow(ks)
                                nc.tensor.matmul(
                                    sp[:, off1 : off1 + w],
                                    kt_sb[:, ks * P : (ks + 1) * P],
                                    qrow[:, q0 + off : q0 + 512],
                                    start=True,
                                    stop=True,
                                )
                                off1 += w
                            sps[g] = sp

                        s_issue(0)
                        if ng > 1:
                            s_issue(1)
                        for g in range(ng):
                            if g + 2 < ng:
                                s_issue(g + 2)
                            # rowsum matmul for the PREVIOUS group's pair:
                            # emitted before this group's PV/direct matmuls
                            # so pair 0 (start=True) is always rp_'s first
                            # writer; its DVE add has had ~a full group to
                            # finish.
                            if g >= 1 and pend_pairs[g - 1] is not None:
                                pr, poff = pend_pairs[g - 1]
                                nc.tensor.matmul(
                                    rp_[:, poff:512],
                                    ones_sb[:],
                                    pr[:, poff:512],
                                    start=(g - 1 == 0),
                                    stop=False,
                                )
                                pend_pairs[g - 1] = None
                            # o_proj filler for the previous q-chunk
                            credit += ucount / slots
                            while credit >= 1.0 and units:
                                o_unit(*units.pop(0), o_ps, o_queues)
                                credit -= 1.0
                            sp = sps[g]
                            pt = pt_pool.tile([P, 1024], bf16, name="ptile",
                                              tag="pt")
                            subs = (2 * g, 2 * g + 1)
                            (offA, wA), (offB, wB) = ow(subs[0]), ow(subs[1])
                            wsum = wA + wB
                            nc.scalar.activation(
                                pt[:, 0:wsum], sp[:, 0:wsum], Exp, scale=SCALE
                            )
                            off1 = 0
                            for ks in subs:
                                w = ow(ks)[1]
                                if ks - (nks - 4) >= 0:
                                    # causal triangle on the first 128 cols
                                    # of the narrowed slice
                                    sl = pt[:, off1 : off1 + P]
                                    nc.vector.tensor_tensor(sl, sl, tri_sb[:],
                                                            MULT)
                                off1 += w
                            last_group = g == ng - 1
                            if not last_group:
                                # pair-reduce the two subtiles on DVE (bf16,
                                # one rounding per element); the rowsum
                                # matmul on the pair streams half the cols.
                                pair = pair_pool.tile([P, 512], bf16,
                                                      name="pair", tag="pair")
                                if offB > offA:
                                    # diagonal pair: [offA:offB] has only A
                                    nc.vector.tensor_copy(
                                        pair[:, offA:offB],
                                        pt[:, 0 : offB - offA],
                                    )
                                    nc.vector.tensor_tensor(
                                        pair[:, offB:512],
                                        pt[:, offB - offA : wA],
                                        pt[:, wA : wA + wB],
                                        ADD,
                                    )
                                else:
                                    nc.vector.tensor_tensor(
                                        pair[:, 0:512],
                                        pt[:, 0:512],
                                        pt[:, 512:1024],
                                        ADD,
                                    )
                                pend_pairs[g] = (pair, offA)
                            off1 = 0
                            for ks in subs:
                                off, w = ow(ks)
                                first, last = ks == 0, ks == nks - 1
                                prhs = pt[:, off1 : off1 + w]
                                off1 += w
                                nc.tensor.matmul(
                                    yp[:, off : off + w],
                                    v_sb[:, ks, :],
                                    prhs,
                                    start=first,
                                    stop=last,
                                )
                                if last_group:
                                    # final (diagonal) group: direct rowsum
                                    # matmuls (executed after pair 0's
                                    # start=True matmul) so nothing is
                                    # deferred across the head boundary.
                                    nc.tensor.matmul(
                                        rp_[:, off : off + w],
                                        ones_sb[:],
                                        prhs,
                                        start=False,
                                        stop=(ks == nks - 1),
                                    )
                        # 1/rowsum (~18 bits; rowsum >= 1 so no edge cases)
                        rinv = nrm_pool.tile([P, 512], f32, name="rinv",
                                             tag="rinv")
                        nc.vector.reciprocal_approx_fast(rinv[:], rp_[:])
                        nc.vector.tensor_tensor(
                            y_sb[h][:, q0 : q0 + 512], yp[:], rinv[:], MULT
                        )
                    # drain any leftover filler units of the previous chunk
                    for u in units:
                        o_unit(*u, o_ps, o_queues)
            # attention PSUM pools closed: 6 banks free. o_proj tail for the
            # last q-chunk runs from a 4-deep PSUM pool (pure matmul stream;
            # evacuation fully hidden), stores on sync/scalar only (gpsimd
            # issues nothing this late - its software-DGE drain is ~7.6us).
            tail_queues = (nc.sync, nc.scalar)
            with tc.tile_pool(name="o_tail_ps", bufs=4, space="PSUM") as o_tail:
                for u in make_units(NQC - 1):
                    o_unit(*u, o_tail, tail_queues)

    nc.compile()
    return nc


def _bf16(a):
    import ml_dtypes

    return np.ascontiguousarray(np.asarray(a, dtype=np.float32)).astype(
        ml_dtypes.bfloat16
    )


def make_in_maps(x, wq, wk, wv, wo, T=T_FULL):
    """Per-core input dicts for run_bass_kernel_spmd."""
    cosT, sinT = _rope_tables(T)
    tri = _tri128()
    onesm = np.ones((P, P), dtype=np.float32)
    rotm = _rot_lhsT()

    xts = [_bf16(x[b].T) for b in range(B)]
    cosT, sinT, tri, onesm, rotm = map(_bf16, (cosT, sinT, tri, onesm, rotm))
    in_maps = []
    for core in range(NCORES):
        b, g = core // 4, core % 4
        wkv = np.concatenate(
            (wk[:, D * g : D * (g + 1)], wv[:, D * g : D * (g + 1)]), axis=1
        )
        in_maps.append(
            {
                "xt": xts[b],
                "wq": _bf16(wq[:, 512 * g : 512 * (g + 1)]),
                "wkv": _bf16(wkv),
                "wo": _bf16(wo[512 * g : 512 * (g + 1), :]),
                "cosT": cosT,
                "sinT": sinT,
                "trim": tri,
                "onesm": onesm,
                "rotm": rotm,
            }
        )
    return in_maps


_NC_CACHE = {}


def _get_nc(T=T_FULL):
    if T not in _NC_CACHE:
        _NC_CACHE[T] = build_nc(T)
    return _NC_CACHE[T]


def run(inputs, trace=False):
    """Run on 8 NeuronCores. Returns (full_output, BassKernelResults)."""
    from concourse.bass_utils import run_bass_kernel_spmd

    x = np.asarray(inputs["x"], dtype=np.float32)
    in_maps = make_in_maps(
        x,
        np.asarray(inputs["wq"], dtype=np.float32),
        np.asarray(inputs["wk"], dtype=np.float32),
        np.asarray(inputs["wv"], dtype=np.float32),
        np.asarray(inputs["wo"], dtype=np.float32),
    )
    nc = _get_nc()
    res = run_bass_kernel_spmd(nc, in_maps, list(range(NCORES)), trace=trace)
    outs = res.results
    full = np.zeros((B, T_FULL, C_DIM), dtype=np.float32)
    for core in range(NCORES):
        full[core // 4] += np.asarray(outs[core]["out"], dtype=np.float32)
    return full, res


def kernel(**inputs):
    full, _ = run(inputs, trace=False)
    return full


# revision 22
# speedup vs baseline: 1.0327x; 1.0327x over previous
"""Trainium2 Bass kernel for causal self-attention with GQA + RoPE.

Model: B=2, T=2048, C=2048, H=16 query heads, H_KV=4 kv heads, D=128.

Sharding (8 NeuronCores, pure SPMD, no collectives):
  core i -> batch b = i // 4, kv-group g = i % 4
            (query heads 4g..4g+3, kv head g, all T positions of batch b).
  Every core runs an identical program; only input data differs.
  o_proj is computed against the row-slice wo[512g:512(g+1), :], giving a
  partial [T, C] output per core; the sum over the 4 cores of each batch
  (the tensor-parallel all-reduce) is done on the host in numpy.

v3 (on top of the bf16 v2 baseline):
  - rowsum pair-reduce: adjacent 128-wide k-subtiles of exp(S) are summed
    pairwise on DVE (bf16 tensor_tensor, 2x mode); the ones-matmul rowsum
    then streams half the columns (PE 29us -> 16us). Cross-pair
    accumulation stays in PSUM fp32 (the single bf16 pair-add rounds
    independently per element -> rowsum error ~2^-9/sqrt(1024), trivial).
    The final (diagonal) group of each head keeps direct rowsum matmuls
    so no cross-head deferral is needed.
  - o_proj PSUM evacuation alternates vector/scalar (was all-vector, which
    serialized the o_unit pipeline through one engine).
  - tail restructure: the attention-only PSUM pools close before the last
    chunk's o_proj, freeing 6 banks; the tail runs from a 4-deep PSUM pool
    with stores on sync/scalar. gpsimd (slow ~7.6us software-DGE drain)
    issues no DMA after mid-attention.

v4 (v4.1 after the big-HWDGE regression):
  - wk|wv concatenated host-side into one wkv tensor (one descriptor per
    cc on gpsimd's software-DGE queue, which issues at ~0.75us/descriptor
    but keeps pace with per-cc K/V consumption).
  - phase-0 x/wq loads as per-cc singles, cc-interleaved across the two
    hardware-DGE queues (sync/scalar) in consumption order. (Big grouped
    transfers on the HW-DGE rings measured ~35GB/s - they do NOT stripe
    across SDMA engines - while a pipelined stream of 128KB descriptors
    sustains ~90GB/s/queue. gpsimd's SWDGE handles grouped transfers
    fine, so bulk non-critical bytes - rope tables, wo - go there.)
  - x for q-chunks 1..3 prefetched one chunk ahead (8 slices into a
    double-buffered tile, alternating sync/scalar).
  - rope restructure: the six PSUM evacuations of a chunk's projections
    are emitted back-to-back at chunk end, split scalar/vector; the rope
    rotate-matmuls + V transposes of chunk qc are interleaved into chunk
    qc+1's projection stream (the PE never waits on the evac/TT chain);
    the rotate/transpose PSUM tag is double-buffered. This removes the
    ~2-3us PE stall at every chunk boundary that also re-throttled the
    PE clock (HAM) mid-kernel.
  - the LAST chunk's rope/V-transpose is deferred into attention chunk 0
    as PE filler (rotate-PSUM from the then-idle o_proj pool), removing
    the phase-transition stall cluster.
"""

import math
import os

import numpy as np

os.environ.setdefault("MYCRO_LOCAL_CACHE", "1")

P = 128
D = 128
H = 16
H_KV = 4
GQ = H // H_KV  # 4 query heads per kv head (= per core)
B = 2
T_FULL = 2048
C_DIM = 2048
NCORES = 8
ROPE_BASE = 10000.0


def _rope_tables(T):
    inv_freq = 1.0 / (ROPE_BASE ** (np.arange(0, D, 2, dtype=np.float32) / D))
    t = np.arange(T, dtype=np.float32)
    freqs = np.outer(t, inv_freq)  # [T, D/2]
    emb = np.concatenate((freqs, freqs), axis=-1)  # [T, D]
    return (
        np.ascontiguousarray(np.cos(emb).T.astype(np.float32)),  # [D, T]
        np.ascontiguousarray(np.sin(emb).T.astype(np.float32)),
    )


def _rot_lhsT():
    # rotate_half(q) = R @ q with R[d, d+64] = -1 (d < 64), R[d, d-64] = +1.
    # matmul computes lhsT.T @ rhs, so pass lhsT = R^T.
    R = np.zeros((D, D), dtype=np.float32)
    half = D // 2
    R[np.arange(half), np.arange(half) + half] = -1.0
    R[np.arange(half) + half, np.arange(half)] = 1.0
    return np.ascontiguousarray(R.T)


def _tri128():
    # tri[k, j] = 1 if j >= k else 0: the in-subtile causal triangle after
    # diagonal narrowing (column j of a narrowed diag slice is q = 128m + j,
    # row k is k_local; valid iff j >= k).
    k = np.arange(P)
    return (k[None, :] >= k[:, None]).astype(np.float32)


def build_nc(T=T_FULL):
    """Build the per-core Bass/Tile program (identical across cores)."""
    from contextlib import ExitStack

    import concourse.mybir as mybir
    import concourse.tile as tile
    from concourse import bacc
    from concourse.masks import make_identity

    f32 = mybir.dt.float32
    bf16 = mybir.dt.bfloat16
    Exp = mybir.ActivationFunctionType.Exp
    MULT = mybir.AluOpType.mult
    ADD = mybir.AluOpType.add
    SCALE = 1.0 / math.sqrt(D)

    NCC = C_DIM // P  # 16 contraction chunks
    NQC = T // 512  # projection / attention q-chunks (512-wide)
    NCT = C_DIM // 512  # o_proj column tiles
    NKB = T // P  # 128-wide k subtiles

    nc = bacc.Bacc(
        "TRN2",
        target_bir_lowering=False,
        debug=False,
        num_devices=NCORES,
    )

    xt = nc.dram_tensor("xt", [C_DIM, T], bf16, kind="ExternalInput").ap()
    wq = nc.dram_tensor("wq", [C_DIM, GQ * D], bf16, kind="ExternalInput").ap()
    wkv = nc.dram_tensor("wkv", [C_DIM, 2 * D], bf16, kind="ExternalInput").ap()
    wo = nc.dram_tensor("wo", [GQ * D, C_DIM], bf16, kind="ExternalInput").ap()
    cosT = nc.dram_tensor("cosT", [D, T], bf16, kind="ExternalInput").ap()
    sinT = nc.dram_tensor("sinT", [D, T], bf16, kind="ExternalInput").ap()
    trim = nc.dram_tensor("trim", [P, P], bf16, kind="ExternalInput").ap()
    onesm = nc.dram_tensor("onesm", [P, P], bf16, kind="ExternalInput").ap()
    rotm = nc.dram_tensor("rotm", [P, P], bf16, kind="ExternalInput").ap()
    out = nc.dram_tensor("out", [T, C_DIM], bf16, kind="ExternalOutput").ap()

    with tile.TileContext(nc) as tc, ExitStack() as ctx:
        const = ctx.enter_context(tc.tile_pool(name="const", bufs=1))
        acts = ctx.enter_context(tc.tile_pool(name="acts", bufs=1))

        wq_r = wq.rearrange("(cc p) n -> p cc n", p=P)
        wkv_r = wkv.rearrange("(cc p) n -> p cc n", p=P)
        xt_r = xt.rearrange("(cc p) t -> p cc t", p=P)
        wo_r = wo.rearrange("(h p) (ct n) -> p h ct n", p=P, n=512)

        ones_sb = const.tile([P, P], bf16)
        rot_sb = const.tile([P, P], bf16)
        ident = const.tile([P, P], bf16)
        tri_sb = const.tile([P, P], bf16)

        # long-lived activations (all bf16: ~60KB/partition total)
        qt_sb = [acts.tile([P, T], bf16, name=f"qt{h}") for h in range(GQ)]
        kt_sb = acts.tile([P, T], bf16, name="kt")
        v_sb = acts.tile([P, NKB, D], bf16, name="vnat")
        y_sb = [acts.tile([P, T], bf16, name=f"yt{h}") for h in range(GQ)]
        wo_sb = acts.tile([P, GQ, NCT, 512], bf16, name="wo_sb")
        cos_sb = acts.tile([P, T], bf16, name="cos_sb")
        sin_sb = acts.tile([P, T], bf16, name="sin_sb")
        # last-chunk projection evacuations, consumed in phase 2
        rawL = [acts.tile([P, 512], bf16, name=f"rawL{i}") for i in range(6)]

        # ---------------- phase 1: projections + rope ----------------
        with (
            tc.tile_pool(name="pwts", bufs=1) as wpool,
            tc.tile_pool(name="xts", bufs=4) as xt_pool,
            tc.tile_pool(name="rope_t", bufs=1) as rope_pool,
            tc.tile_pool(name="proj_ps", bufs=1, space="PSUM") as proj_ps,
            tc.tile_pool(name="aux_ps", bufs=1, space="PSUM") as aux_ps,
            tc.tile_pool(name="ptmp", bufs=2) as ptmp,
        ):
            wq_sb = wpool.tile([P, NCC, GQ * D], bf16)
            wkv_sb = wpool.tile([P, NCC, 2 * D], bf16)
            lead_xs = xt_pool.tile([P, NCC, 512], bf16, tag="xlead",
                               name="lead_xs", bufs=1)

            # identity first: two cheap gpsimd ops, then gpsimd is free to
            # issue DMA descriptors.
            make_identity(nc, ident)

            # Phase-0 loads: x/wq as per-cc singles, cc-interleaved across
            # the two HW-DGE queues (each pipelines 128KB descriptors at
            # ~90GB/s; larger transfers on these rings are NOT faster -
            # they don't stripe across SDMA engines). wkv singles and all
            # bulk non-urgent bytes go on gpsimd's SWDGE queue.
            for cc in range(NCC):
                qa, qb = (nc.sync, nc.scalar) if cc % 2 == 0 else (
                    nc.scalar, nc.sync)
                qa.dma_start(lead_xs[:, cc, :], xt_r[:, cc, 0:512])
                qb.dma_start(wq_sb[:, cc, :], wq_r[:, cc, :])
                nc.gpsimd.dma_start(wkv_sb[:, cc, :], wkv_r[:, cc, :])
            # rope tables / consts on gpsimd (rotm + chunk-0 tables needed
            # from ~33us; the rest later).
            nc.gpsimd.dma_start(rot_sb[:], rotm)
            nc.gpsimd.dma_start(cos_sb[:, 0:512], cosT[:, 0:512])
            nc.gpsimd.dma_start(sin_sb[:, 0:512], sinT[:, 0:512])
            nc.gpsimd.dma_start(tri_sb[:], trim)
            nc.gpsimd.dma_start(ones_sb[:], onesm)
            nc.gpsimd.dma_start(cos_sb[:, 512:T], cosT[:, 512:T])
            nc.gpsimd.dma_start(sin_sb[:, 512:T], sinT[:, 512:T])
            # wo preload (needed ~115us) as two 1MB transfers.
            nc.gpsimd.dma_start(wo_sb[:, 0:2, :, :], wo_r[:, 0:2, :, :])
            nc.gpsimd.dma_start(wo_sb[:, 2:4, :, :], wo_r[:, 2:4, :, :])
            # warm the ACT exp table set during the initial DMA wait
            warm = ptmp.tile([P, 1], f32, name="warm", tag="warm")
            nc.scalar.activation(warm[:], warm[:], Exp)

            def rot_tt(raw, dst, cosq, sinq):
                # dst = raw*cos + (R raw)*sin
                rp = aux_ps.tile([P, 512], f32, name="rotp", tag="rotp",
                                 bufs=2)
                nc.tensor.matmul(rp[:], rot_sb[:], raw[:], start=True,
                                 stop=True)
                nc.vector.tensor_tensor(dst, raw[:], cosq, MULT)
                t2 = ptmp.tile([P, 512], bf16, name="rt2", tag="rt2")
                nc.vector.tensor_tensor(t2[:], rp[:], sinq, MULT)
                nc.vector.tensor_tensor(dst, dst, t2[:], ADD)

            # pending rope work of the previous chunk, interleaved into the
            # current chunk's projection stream so the PE never waits on
            # the PSUM-evacuation/TT chain.
            pend_rope = None  # (qc_prev, raws[4], rawk, vraw)
            xh_next = None

            for qc in range(NQC):
                q0 = qc * 512
                if qc == 0:
                    xt_all = lead_xs
                else:
                    xt_all = xh_next
                # prefetch next chunk's x, 8 slices alternating queues
                if qc + 1 < NQC:
                    nq0 = (qc + 1) * 512
                    xh = xt_pool.tile([P, NCC, 512], bf16, tag="xh",
                                      name="xh", bufs=2)
                    for xg in range(8):
                        q_ = (nc.sync, nc.scalar)[xg % 2]
                        q_.dma_start(
                            xh[:, 2 * xg : 2 * xg + 2, :],
                            xt_r[:, 2 * xg : 2 * xg + 2, nq0 : nq0 + 512],
                        )
                    xh_next = xh

                qp = [
                    proj_ps.tile([P, 512], f32, name=f"qp{h}", tag=f"qp{h}")
                    for h in range(GQ)
                ]
                kp = proj_ps.tile([P, 512], f32, name="kp", tag="kp")
                vp = proj_ps.tile([P, 512], f32, name="vp", tag="vp")
                for cc in range(NCC):
                    xtile = xt_all[:, cc, :]
                    first, last = cc == 0, cc == NCC - 1
                    for h in range(GQ):
                        nc.tensor.matmul(
                            qp[h][:],
                            wq_sb[:, cc, h * D : (h + 1) * D],
                            xtile,
                            start=first,
                            stop=last,
                        )
                    nc.tensor.matmul(
                        kp[:], wkv_sb[:, cc, 0:D], xtile, start=first,
                        stop=last
                    )
                    nc.tensor.matmul(
                        vp[:], wkv_sb[:, cc, D : 2 * D], xtile, start=first,
                        stop=last
                    )
                    # previous chunk's rope/V-transpose work as filler
                    if pend_rope is not None and cc in (0, 1, 2, 4, 5):
                        pq, raws, rawk, vraw = pend_rope
                        pq0 = pq * 512
                        pcos = cos_sb[:, pq0 : pq0 + 512]
                        psin = sin_sb[:, pq0 : pq0 + 512]
                        if cc == 0:
                            rot_tt(raws[0], qt_sb[0][:, pq0 : pq0 + 512],
                                   pcos, psin)
                            rot_tt(raws[1], qt_sb[1][:, pq0 : pq0 + 512],
                                   pcos, psin)
                        elif cc == 1:
                            rot_tt(raws[2], qt_sb[2][:, pq0 : pq0 + 512],
                                   pcos, psin)
                            rot_tt(raws[3], qt_sb[3][:, pq0 : pq0 + 512],
                                   pcos, psin)
                        elif cc == 2:
                            rot_tt(rawk, kt_sb[:, pq0 : pq0 + 512],
                                   pcos, psin)
                        elif cc in (4, 5):
                            for ks in ((0, 1) if cc == 4 else (2, 3)):
                                tp = aux_ps.tile([P, P], bf16, name="vtrp",
                                                 tag="rotp", bufs=2)
                                nc.tensor.transpose(
                                    tp[:], vraw[:, ks * P : (ks + 1) * P],
                                    ident[:],
                                )
                                nc.vector.tensor_copy(
                                    v_sb[:, pq * 4 + ks, :], tp[:]
                                )

                # end of chunk: evacuate all six projection accumulators
                # into the long-lived rawL tiles, split across scalar and
                # vector so the wave is ~2x faster; the rope matmuls run
                # inside the next chunk's projections (or, for the last
                # chunk, as attention filler in phase 2).
                for h in range(GQ):
                    if h % 2 == 0:
                        nc.scalar.copy(rawL[h][:], qp[h][:])
                    else:
                        nc.vector.tensor_copy(rawL[h][:], qp[h][:])
                nc.scalar.copy(rawL[4][:], kp[:])
                nc.vector.tensor_copy(rawL[5][:], vp[:])
                pend_rope = (qc, rawL[0:4], rawL[4], rawL[5])

        # -------- phase 2: causal attention + interleaved o_proj --------
        with (
            tc.tile_pool(name="pt_pool", bufs=3) as pt_pool,
            tc.tile_pool(name="o_ps", bufs=2, space="PSUM") as o_ps,
            tc.tile_pool(name="nrm", bufs=2) as nrm_pool,
            tc.tile_pool(name="ost", bufs=4) as ost_pool,
            tc.tile_pool(name="pairs", bufs=3) as pair_pool,
        ):
            o_count = [0]
            o_queues = (nc.sync, nc.scalar, nc.gpsimd)
            evac_engines = (nc.vector, nc.scalar)

            def o_unit(aq, ct, qb, ps_pool, store_queues):
                # one o_proj output tile [128 q rows, 512 cols] for chunk aq
                op = ps_pool.tile([P, 512], f32, name="op", tag="op")
                for h in range(GQ):
                    nc.tensor.matmul(
                        op[:],
                        y_sb[h][:, qb * P : (qb + 1) * P],
                        wo_sb[:, h, ct, :],
                        start=(h == 0),
                        stop=(h == GQ - 1),
                    )
                ot = ost_pool.tile([P, 512], bf16, name="ot", tag="ot")
                ev = evac_engines[o_count[0] % 2]
                if ev is nc.scalar:
                    nc.scalar.copy(ot[:], op[:])
                else:
                    nc.vector.tensor_copy(ot[:], op[:])
                oq = store_queues[o_count[0] % len(store_queues)]
                o_count[0] += 1
                oq.dma_start(
                    out[qb * P : (qb + 1) * P, ct * 512 : (ct + 1) * 512],
                    ot[:],
                )

            def make_units(aq):
                return [(aq, ct, qb) for ct in range(NCT)
                        for qb in range(4 * aq, 4 * aq + 4)]

            # deferred rope/V-transpose of the last projection chunk,
            # executed as PE filler inside attention chunk 0 (the o_proj
            # pool is idle there, so its PSUM banks host the rotate
            # matmuls / transposes).
            pq, praws, prawk, pvraw = pend_rope
            pq0 = pq * 512
            pcos = cos_sb[:, pq0 : pq0 + 512]
            psin = sin_sb[:, pq0 : pq0 + 512]

            def d_rot(raw, dst):
                def fn():
                    rp = o_ps.tile([P, 512], f32, name="rpd", tag="op")
                    nc.tensor.matmul(rp[:], rot_sb[:], raw[:], start=True,
                                     stop=True)
                    nc.vector.tensor_tensor(dst, raw[:], pcos, MULT)
                    t2 = nrm_pool.tile([P, 512], bf16, name="rt2d",
                                       tag="rt2d")
                    nc.vector.tensor_tensor(t2[:], rp[:], psin, MULT)
                    nc.vector.tensor_tensor(dst, dst, t2[:], ADD)
                return fn

            def d_vt(ks0, ks1):
                def fn():
                    for ks in (ks0, ks1):
                        tp = o_ps.tile([P, P], bf16, name="vtpd", tag="op")
                        nc.tensor.transpose(
                            tp[:], pvraw[:, ks * P : (ks + 1) * P], ident[:]
                        )
                        nc.vector.tensor_copy(v_sb[:, pq * 4 + ks, :], tp[:])
                return fn

            rope_fill = [d_rot(praws[h], qt_sb[h][:, pq0 : pq0 + 512])
                         for h in range(GQ)]
            rope_fill.append(d_rot(prawk, kt_sb[:, pq0 : pq0 + 512]))
            rope_fill.append(d_vt(0, 1))
            rope_fill.append(d_vt(2, 3))

            with (
                tc.tile_pool(name="s_ps", bufs=2, space="PSUM") as s_ps,
                tc.tile_pool(name="y_ps", bufs=1, space="PSUM") as y_ps,
                tc.tile_pool(name="rs_ps", bufs=1, space="PSUM") as rs_ps,
            ):
                for aq in range(NQC):
                    q0 = aq * 512
                    nks = 4 * (aq + 1)  # 128-wide k subtiles (incl 4 diagonal)
                    ng = nks // 2  # groups of 2 subtiles
                    units = make_units(aq - 1) if aq > 0 else []
                    slots = GQ * ng
                    credit = 0.0
                    ucount = len(units)

                    # narrowed (offset, width) per k-subtile: diagonal subtile
                    # m only covers q >= 128m within the 512-wide chunk.
                    def ow(ks):
                        m = ks - (nks - 4)
                        if m > 0:
                            return 128 * m, 512 - 128 * m
                        return 0, 512

                    for h in range(GQ):
                        qrow = qt_sb[h]
                        yp = y_ps.tile([P, 512], f32, name="yp", tag="yp")
                        rp_ = rs_ps.tile([P, 512], f32, name="rsp", tag="rsp")
                        sps = [None] * ng
                        # pair tiles awaiting their rowsum matmul:
                        # list of (tile, offA) in group order
                        pend_pairs = [None] * ng

                        def s_issue(g):
                            # the two subtiles are packed back to back in the
                            # sp tile ([0:w0], [w0:w0+w1]); w0 is always 256
                            # or 512 so neither matmul output crosses a PSUM
                            # bank.
                            sp = s_ps.tile([P, 1024], f32, name="sp", tag="sp")
                            off1 = 0
                            for ks in (2 * g, 2 * g + 1):
                                off, w = ow(ks)
                                nc.tensor.matmul(
                                    sp[:, off1 : off1 + w],
                                    kt_sb[:, ks * P : (ks + 1) * P],
                                    qrow[:, q0 + off : q0 + 512],
                                    start=True,
                                    stop=True,
                                )
                                off1 += w
                            sps[g] = sp

                        s_issue(0)
                        if ng > 1:
                            s_issue(1)
                        for g in range(ng):
                            if g + 2 < ng:
                                s_issue(g + 2)
                            # rowsum matmul for the PREVIOUS group's pair:
                            # emitted before this group's PV/direct matmuls
                            # so pair 0 (start=True) is always rp_'s first
                            # writer; its DVE add has had ~a full group to
                            # finish.
                            if g >= 1 and pend_pairs[g - 1] is not None:
                                pr, poff = pend_pairs[g - 1]
                                nc.tensor.matmul(
                                    rp_[:, poff:512],
                                    ones_sb[:],
                                    pr[:, poff:512],
                                    start=(g - 1 == 0),
                                    stop=False,
                                )
                                pend_pairs[g - 1] = None
                            # deferred last-chunk rope as PE filler (aq 0)
                            if rope_fill:
                                rope_fill.pop(0)()
                            # o_proj filler for the previous q-chunk
                            credit += ucount / slots
                            while credit >= 1.0 and units:
                                o_unit(*units.pop(0), o_ps, o_queues)
                                credit -= 1.0
                            sp = sps[g]
                            pt = pt_pool.tile([P, 1024], bf16, name="ptile",
                                              tag="pt")
                            subs = (2 * g, 2 * g + 1)
                            (offA, wA), (offB, wB) = ow(subs[0]), ow(subs[1])
                            wsum = wA + wB
                            nc.scalar.activation(
                                pt[:, 0:wsum], sp[:, 0:wsum], Exp, scale=SCALE
                            )
                            off1 = 0
                            for ks in subs:
                                w = ow(ks)[1]
                                if ks - (nks - 4) >= 0:
                                    # causal triangle on the first 128 cols
                                    # of the narrowed slice
                                    sl = pt[:, off1 : off1 + P]
                                    nc.vector.tensor_tensor(sl, sl, tri_sb[:],
                                                            MULT)
                                off1 += w
                            last_group = g == ng - 1
                            if not last_group:
                                # pair-reduce the two subtiles on DVE (bf16,
                                # one rounding per element); the rowsum
                                # matmul on the pair streams half the cols.
                                pair = pair_pool.tile([P, 512], bf16,
                                                      name="pair", tag="pair")
                                if offB > offA:
                                    # diagonal pair: [offA:offB] has only A
                                    nc.vector.tensor_copy(
                                        pair[:, offA:offB],
                                        pt[:, 0 : offB - offA],
                                    )
                                    nc.vector.tensor_tensor(
                                        pair[:, offB:512],
                                        pt[:, offB - offA : wA],
                                        pt[:, wA : wA + wB],
                                        ADD,
                                    )
                                else:
                                    nc.vector.tensor_tensor(
                                        pair[:, 0:512],
                                        pt[:, 0:512],
                                        pt[:, 512:1024],
                                        ADD,
                                    )
                                pend_pairs[g] = (pair, offA)
                            off1 = 0
                            for ks in subs:
                                off, w = ow(ks)
                                first, last = ks == 0, ks == nks - 1
                                prhs = pt[:, off1 : off1 + w]
                                off1 += w
                                nc.tensor.matmul(
                                    yp[:, off : off + w],
                                    v_sb[:, ks, :],
                                    prhs,
                                    start=first,
                                    stop=last,
                                )
                                if last_group:
                                    # final (diagonal) group: direct rowsum
                                    # matmuls (executed after pair 0's
                                    # start=True matmul) so nothing is
                                    # deferred across the head boundary.
                                    nc.tensor.matmul(
                                        rp_[:, off : off + w],
                                        ones_sb[:],
                                        prhs,
                                        start=False,
                                        stop=(ks == nks - 1),
                                    )
                        # 1/rowsum (~18 bits; rowsum >= 1 so no edge cases)
                        rinv = nrm_pool.tile([P, 512], f32, name="rinv",
                                             tag="rinv")
                        nc.vector.reciprocal_approx_fast(rinv[:], rp_[:])
                        nc.vector.tensor_tensor(
                            y_sb[h][:, q0 : q0 + 512], yp[:], rinv[:], MULT
                        )
                    # drain any leftover filler units of the previous chunk
                    for u in units:
                        o_unit(*u, o_ps, o_queues)
            # attention PSUM pools closed: 6 banks free. o_proj tail for the
            # last q-chunk runs from a 4-deep PSUM pool (pure matmul stream;
            # evacuation fully hidden), stores on sync/scalar only (gpsimd
            # issues nothing this late - its software-DGE drain is ~7.6us).
            tail_queues = (nc.sync, nc.scalar)
            with tc.tile_pool(name="o_tail_ps", bufs=4, space="PSUM") as o_tail:
                for u in make_units(NQC - 1):
                    o_unit(*u, o_tail, tail_queues)

    nc.compile()
    return nc


def _bf16(a):
    import ml_dtypes

    return np.ascontiguousarray(np.asarray(a, dtype=np.float32)).astype(
        ml_dtypes.bfloat16
    )


def make_in_maps(x, wq, wk, wv, wo, T=T_FULL):
    """Per-core input dicts for run_bass_kernel_spmd."""
    cosT, sinT = _rope_tables(T)
    tri = _tri128()
    onesm = np.ones((P, P), dtype=np.float32)
    rotm = _rot_lhsT()

    xts = [_bf16(x[b].T) for b in range(B)]
    cosT, sinT, tri, onesm, rotm = map(_bf16, (cosT, sinT, tri, onesm, rotm))
    in_maps = []
    for core in range(NCORES):
        b, g = core // 4, core % 4
        wkv = np.concatenate(
            (wk[:, D * g : D * (g + 1)], wv[:, D * g : D * (g + 1)]), axis=1
        )
        in_maps.append(
            {
                "xt": xts[b],
                "wq": _bf16(wq[:, 512 * g : 512 * (g + 1)]),
                "wkv": _bf16(wkv),
                "wo": _bf16(wo[512 * g : 512 * (g + 1), :]),
                "cosT": cosT,
                "sinT": sinT,
                "trim": tri,
                "onesm": onesm,
                "rotm": rotm,
            }
        )
    return in_maps


_NC_CACHE = {}


def _get_nc(T=T_FULL):
    if T not in _NC_CACHE:
        _NC_CACHE[T] = build_nc(T)
    return _NC_CACHE[T]


def run(inputs, trace=False):
    """Run on 8 NeuronCores. Returns (full_output, BassKernelResults)."""
    from concourse.bass_utils import run_bass_kernel_spmd

    x = np.asarray(inputs["x"], dtype=np.float32)
    in_maps = make_in_maps(
        x,
        np.asarray(inputs["wq"], dtype=np.float32),
        np.asarray(inputs["wk"], dtype=np.float32),
        np.asarray(inputs["wv"], dtype=np.float32),
        np.asarray(inputs["wo"], dtype=np.float32),
    )
    nc = _get_nc()
    res = run_bass_kernel_spmd(nc, in_maps, list(range(NCORES)), trace=trace)
    outs = res.results
    full = np.zeros((B, T_FULL, C_DIM), dtype=np.float32)
    for core in range(NCORES):
        full[core // 4] += np.asarray(outs[core]["out"], dtype=np.float32)
    return full, res


def kernel(**inputs):
    full, _ = run(inputs, trace=False)
    return full


# revision 25
# speedup vs baseline: 1.0473x; 1.0141x over previous
"""Trainium2 Bass kernel for causal self-attention with GQA + RoPE.

Model: B=2, T=2048, C=2048, H=16 query heads, H_KV=4 kv heads, D=128.

Sharding (8 NeuronCores, pure SPMD, no collectives):
  core i -> batch b = i // 4, kv-group g = i % 4
            (query heads 4g..4g+3, kv head g, all T positions of batch b).
  Every core runs an identical program; only input data differs.
  o_proj is computed against the row-slice wo[512g:512(g+1), :], giving a
  partial [T, C] output per core; the sum over the 4 cores of each batch
  (the tensor-parallel all-reduce) is done on the host in numpy.

v3 (on top of the bf16 v2 baseline):
  - rowsum pair-reduce: adjacent 128-wide k-subtiles of exp(S) are summed
    pairwise on DVE (bf16 tensor_tensor, 2x mode); the ones-matmul rowsum
    then streams half the columns (PE 29us -> 16us). Cross-pair
    accumulation stays in PSUM fp32 (the single bf16 pair-add rounds
    independently per element -> rowsum error ~2^-9/sqrt(1024), trivial).
    The final (diagonal) group of each head keeps direct rowsum matmuls
    so no cross-head deferral is needed.
  - o_proj PSUM evacuation alternates vector/scalar (was all-vector, which
    serialized the o_unit pipeline through one engine).
  - tail restructure: the attention-only PSUM pools close before the last
    chunk's o_proj, freeing 6 banks; the tail runs from a 4-deep PSUM pool
    with stores on sync/scalar. gpsimd (slow ~7.6us software-DGE drain)
    issues no DMA after mid-attention.

v4 (v4.1 after the big-HWDGE regression):
  - wk|wv concatenated host-side into one wkv tensor (one descriptor per
    cc on gpsimd's software-DGE queue, which issues at ~0.75us/descriptor
    but keeps pace with per-cc K/V consumption).
  - phase-0 x/wq loads as per-cc singles, cc-interleaved across the two
    hardware-DGE queues (sync/scalar) in consumption order. (Big grouped
    transfers on the HW-DGE rings measured ~35GB/s - they do NOT stripe
    across SDMA engines - while a pipelined stream of 128KB descriptors
    sustains ~90GB/s/queue. gpsimd's SWDGE handles grouped transfers
    fine, so bulk non-critical bytes - rope tables, wo - go there.)
  - x for q-chunks 1..3 prefetched one chunk ahead (8 slices into a
    double-buffered tile, alternating sync/scalar).
  - rope restructure: the six PSUM evacuations of a chunk's projections
    are emitted back-to-back at chunk end, split scalar/vector; the rope
    rotate-matmuls + V transposes of chunk qc are interleaved into chunk
    qc+1's projection stream (the PE never waits on the evac/TT chain);
    the rotate/transpose PSUM tag is double-buffered. This removes the
    ~2-3us PE stall at every chunk boundary that also re-throttled the
    PE clock (HAM) mid-kernel.
  - the LAST chunk's rope/V-transpose is deferred into attention chunk 0
    as PE filler (rotate-PSUM from the then-idle o_proj pool), removing
    the phase-transition stall cluster.
"""

import math
import os

import numpy as np

os.environ.setdefault("MYCRO_LOCAL_CACHE", "1")

P = 128
D = 128
H = 16
H_KV = 4
GQ = H // H_KV  # 4 query heads per kv head (= per core)
B = 2
T_FULL = 2048
C_DIM = 2048
NCORES = 8
ROPE_BASE = 10000.0


def _rope_tables(T):
    inv_freq = 1.0 / (ROPE_BASE ** (np.arange(0, D, 2, dtype=np.float32) / D))
    t = np.arange(T, dtype=np.float32)
    freqs = np.outer(t, inv_freq)  # [T, D/2]
    emb = np.concatenate((freqs, freqs), axis=-1)  # [T, D]
    return (
        np.ascontiguousarray(np.cos(emb).T.astype(np.float32)),  # [D, T]
        np.ascontiguousarray(np.sin(emb).T.astype(np.float32)),
    )


def _rot_lhsT():
    # rotate_half(q) = R @ q with R[d, d+64] = -1 (d < 64), R[d, d-64] = +1.
    # matmul computes lhsT.T @ rhs, so pass lhsT = R^T.
    R = np.zeros((D, D), dtype=np.float32)
    half = D // 2
    R[np.arange(half), np.arange(half) + half] = -1.0
    R[np.arange(half) + half, np.arange(half)] = 1.0
    return np.ascontiguousarray(R.T)


def _tri128():
    # tri[k, j] = 1 if j >= k else 0: the in-subtile causal triangle after
    # diagonal narrowing (column j of a narrowed diag slice is q = 128m + j,
    # row k is k_local; valid iff j >= k).
    k = np.arange(P)
    return (k[None, :] >= k[:, None]).astype(np.float32)


def build_nc(T=T_FULL):
    """Build the per-core Bass/Tile program (identical across cores)."""
    from contextlib import ExitStack

    import concourse.mybir as mybir
    import concourse.tile as tile
    from concourse import bacc
    from concourse.masks import make_identity

    f32 = mybir.dt.float32
    bf16 = mybir.dt.bfloat16
    Exp = mybir.ActivationFunctionType.Exp
    MULT = mybir.AluOpType.mult
    ADD = mybir.AluOpType.add
    SCALE = 1.0 / math.sqrt(D)

    NCC = C_DIM // P  # 16 contraction chunks
    NQC = T // 512  # projection / attention q-chunks (512-wide)
    NCT = C_DIM // 512  # o_proj column tiles
    NKB = T // P  # 128-wide k subtiles

    nc = bacc.Bacc(
        "TRN2",
        target_bir_lowering=False,
        debug=False,
        num_devices=NCORES,
    )

    xt = nc.dram_tensor("xt", [C_DIM, T], bf16, kind="ExternalInput").ap()
    wq = nc.dram_tensor("wq", [C_DIM, GQ * D], bf16, kind="ExternalInput").ap()
    wkv = nc.dram_tensor("wkv", [C_DIM, 2 * D], bf16, kind="ExternalInput").ap()
    wo = nc.dram_tensor("wo", [GQ * D, C_DIM], bf16, kind="ExternalInput").ap()
    cosT = nc.dram_tensor("cosT", [D, T], bf16, kind="ExternalInput").ap()
    sinT = nc.dram_tensor("sinT", [D, T], bf16, kind="ExternalInput").ap()
    trim = nc.dram_tensor("trim", [P, P], bf16, kind="ExternalInput").ap()
    onesm = nc.dram_tensor("onesm", [P, P], bf16, kind="ExternalInput").ap()
    rotm = nc.dram_tensor("rotm", [P, P], bf16, kind="ExternalInput").ap()
    out = nc.dram_tensor("out", [T, C_DIM], bf16, kind="ExternalOutput").ap()

    with tile.TileContext(nc) as tc, ExitStack() as ctx:
        const = ctx.enter_context(tc.tile_pool(name="const", bufs=1))
        acts = ctx.enter_context(tc.tile_pool(name="acts", bufs=1))

        wq_r = wq.rearrange("(cc p) n -> p cc n", p=P)
        wkv_r = wkv.rearrange("(cc p) n -> p cc n", p=P)
        xt_r = xt.rearrange("(cc p) t -> p cc t", p=P)
        wo_r = wo.rearrange("(h p) (ct n) -> p h ct n", p=P, n=512)

        ones_sb = const.tile([P, P], bf16)
        rot_sb = const.tile([P, P], bf16)
        ident = const.tile([P, P], bf16)
        tri_sb = const.tile([P, P], bf16)

        # long-lived activations (all bf16: ~60KB/partition total)
        qt_sb = [acts.tile([P, T], bf16, name=f"qt{h}") for h in range(GQ)]
        kt_sb = acts.tile([P, T], bf16, name="kt")
        v_sb = acts.tile([P, NKB, D], bf16, name="vnat")
        y_sb = [acts.tile([P, T], bf16, name=f"yt{h}") for h in range(GQ)]
        wo_sb = acts.tile([P, GQ, NCT, 512], bf16, name="wo_sb")
        cos_sb = acts.tile([P, T], bf16, name="cos_sb")
        sin_sb = acts.tile([P, T], bf16, name="sin_sb")
        # last-chunk projection evacuations, consumed in phase 2
        rawL = [acts.tile([P, 512], bf16, name=f"rawL{i}") for i in range(6)]

        # ---------------- phase 1: projections + rope ----------------
        with (
            tc.tile_pool(name="pwts", bufs=1) as wpool,
            tc.tile_pool(name="xts", bufs=4) as xt_pool,
            tc.tile_pool(name="rope_t", bufs=1) as rope_pool,
            tc.tile_pool(name="proj_ps", bufs=1, space="PSUM") as proj_ps,
            tc.tile_pool(name="aux_ps", bufs=1, space="PSUM") as aux_ps,
            tc.tile_pool(name="ptmp", bufs=2) as ptmp,
        ):
            wq_sb = wpool.tile([P, NCC, GQ * D], bf16)
            wkv_sb = wpool.tile([P, NCC, 2 * D], bf16)
            lead_xs = xt_pool.tile([P, NCC, 512], bf16, tag="xlead",
                               name="lead_xs", bufs=1)

            # identity first: two cheap gpsimd ops, then gpsimd is free to
            # issue DMA descriptors.
            make_identity(nc, ident)

            # Phase-0 loads: x/wq as per-cc singles, cc-interleaved across
            # the two HW-DGE queues (each pipelines 128KB descriptors at
            # ~90GB/s; larger transfers on these rings are NOT faster -
            # they don't stripe across SDMA engines). wkv singles and all
            # bulk non-urgent bytes go on gpsimd's SWDGE queue.
            for cc in range(NCC):
                qa, qb = (nc.sync, nc.scalar) if cc % 2 == 0 else (
                    nc.scalar, nc.sync)
                qa.dma_start(lead_xs[:, cc, :], xt_r[:, cc, 0:512])
                qb.dma_start(wq_sb[:, cc, :], wq_r[:, cc, :])
                nc.gpsimd.dma_start(wkv_sb[:, cc, :], wkv_r[:, cc, :])
            # rope tables / consts on gpsimd (rotm + chunk-0 tables needed
            # from ~33us; the rest later).
            nc.gpsimd.dma_start(rot_sb[:], rotm)
            nc.gpsimd.dma_start(cos_sb[:, 0:512], cosT[:, 0:512])
            nc.gpsimd.dma_start(sin_sb[:, 0:512], sinT[:, 0:512])
            nc.gpsimd.dma_start(tri_sb[:], trim)
            nc.gpsimd.dma_start(ones_sb[:], onesm)
            nc.gpsimd.dma_start(cos_sb[:, 512:T], cosT[:, 512:T])
            nc.gpsimd.dma_start(sin_sb[:, 512:T], sinT[:, 512:T])
            # wo preload (needed ~115us) as two 1MB transfers.
            nc.gpsimd.dma_start(wo_sb[:, 0:2, :, :], wo_r[:, 0:2, :, :])
            nc.gpsimd.dma_start(wo_sb[:, 2:4, :, :], wo_r[:, 2:4, :, :])
            # warm the ACT exp table set during the initial DMA wait
            warm = ptmp.tile([P, 1], f32, name="warm", tag="warm")
            nc.scalar.activation(warm[:], warm[:], Exp)

            def rot_tt(raw, dst, cosq, sinq):
                # dst = raw*cos + (R raw)*sin
                rp = aux_ps.tile([P, 512], f32, name="rotp", tag="rotp",
                                 bufs=2)
                nc.tensor.matmul(rp[:], rot_sb[:], raw[:], start=True,
                                 stop=True)
                nc.vector.tensor_tensor(dst, raw[:], cosq, MULT)
                t2 = ptmp.tile([P, 512], bf16, name="rt2", tag="rt2")
                nc.vector.tensor_tensor(t2[:], rp[:], sinq, MULT)
                nc.vector.tensor_tensor(dst, dst, t2[:], ADD)

            # pending rope work of the previous chunk, interleaved into the
            # current chunk's projection stream so the PE never waits on
            # the PSUM-evacuation/TT chain.
            pend_rope = None  # (qc_prev, raws[4], rawk, vraw)
            xh_next = None

            for qc in range(NQC):
                q0 = qc * 512
                if qc == 0:
                    xt_all = lead_xs
                else:
                    xt_all = xh_next
                # next chunk's x prefetch: issued one slice per cc during
                # the second half of this chunk (cc 8..15). A dma_start
                # BLOCKS its issuing engine while the HW-DGE ring is full,
                # and everything behind it in that engine's FIFO (e.g. the
                # PSUM evacuation copies) stalls with it - so the posts
                # must go out only as the ring drains, never in a burst
                # up front.
                xh = None
                if qc + 1 < NQC:
                    xh = xt_pool.tile([P, NCC, 512], bf16, tag="xh",
                                      name="xh", bufs=2)

                qp = [
                    proj_ps.tile([P, 512], f32, name=f"qp{h}", tag=f"qp{h}")
                    for h in range(GQ)
                ]
                kp = proj_ps.tile([P, 512], f32, name="kp", tag="kp")
                vp = proj_ps.tile([P, 512], f32, name="vp", tag="vp")
                for cc in range(NCC):
                    xtile = xt_all[:, cc, :]
                    first, last = cc == 0, cc == NCC - 1
                    for h in range(GQ):
                        nc.tensor.matmul(
                            qp[h][:],
                            wq_sb[:, cc, h * D : (h + 1) * D],
                            xtile,
                            start=first,
                            stop=last,
                        )
                    nc.tensor.matmul(
                        kp[:], wkv_sb[:, cc, 0:D], xtile, start=first,
                        stop=last
                    )
                    nc.tensor.matmul(
                        vp[:], wkv_sb[:, cc, D : 2 * D], xtile, start=first,
                        stop=last
                    )
                    if xh is not None and cc >= 8:
                        xg = cc - 8
                        nq0 = (qc + 1) * 512
                        q_ = (nc.sync, nc.scalar)[xg % 2]
                        q_.dma_start(
                            xh[:, 2 * xg : 2 * xg + 2, :],
                            xt_r[:, 2 * xg : 2 * xg + 2, nq0 : nq0 + 512],
                        )
                    # previous chunk's rope/V-transpose work as filler
                    if pend_rope is not None and cc in (0, 1, 2, 4, 5):
                        pq, raws, rawk, vraw = pend_rope
                        pq0 = pq * 512
                        pcos = cos_sb[:, pq0 : pq0 + 512]
                        psin = sin_sb[:, pq0 : pq0 + 512]
                        if cc == 0:
                            rot_tt(raws[0], qt_sb[0][:, pq0 : pq0 + 512],
                                   pcos, psin)
                            rot_tt(raws[1], qt_sb[1][:, pq0 : pq0 + 512],
                                   pcos, psin)
                        elif cc == 1:
                            rot_tt(raws[2], qt_sb[2][:, pq0 : pq0 + 512],
                                   pcos, psin)
                            rot_tt(raws[3], qt_sb[3][:, pq0 : pq0 + 512],
                                   pcos, psin)
                        elif cc == 2:
                            rot_tt(rawk, kt_sb[:, pq0 : pq0 + 512],
                                   pcos, psin)
                        elif cc in (4, 5):
                            for ks in ((0, 1) if cc == 4 else (2, 3)):
                                tp = aux_ps.tile([P, P], bf16, name="vtrp",
                                                 tag="rotp", bufs=2)
                                nc.tensor.transpose(
                                    tp[:], vraw[:, ks * P : (ks + 1) * P],
                                    ident[:],
                                )
                                nc.vector.tensor_copy(
                                    v_sb[:, pq * 4 + ks, :], tp[:]
                                )

                # end of chunk: evacuate all six projection accumulators
                # into the long-lived rawL tiles, split across scalar and
                # vector so the wave is ~2x faster; the rope matmuls run
                # inside the next chunk's projections (or, for the last
                # chunk, as attention filler in phase 2).
                for h in range(GQ):
                    if h % 2 == 0:
                        nc.scalar.copy(rawL[h][:], qp[h][:])
                    else:
                        nc.vector.tensor_copy(rawL[h][:], qp[h][:])
                nc.scalar.copy(rawL[4][:], kp[:])
                nc.vector.tensor_copy(rawL[5][:], vp[:])
                pend_rope = (qc, rawL[0:4], rawL[4], rawL[5])
                xh_next = xh

        # -------- phase 2: causal attention + interleaved o_proj --------
        with (
            tc.tile_pool(name="pt_pool", bufs=3) as pt_pool,
            tc.tile_pool(name="o_ps", bufs=2, space="PSUM") as o_ps,
            tc.tile_pool(name="nrm", bufs=2) as nrm_pool,
            tc.tile_pool(name="ost", bufs=4) as ost_pool,
            tc.tile_pool(name="pairs", bufs=3) as pair_pool,
        ):
            o_count = [0]
            o_queues = (nc.sync, nc.scalar, nc.gpsimd)
            evac_engines = (nc.vector, nc.scalar)

            def o_unit(aq, ct, qb, ps_pool, store_queues):
                # one o_proj output tile [128 q rows, 512 cols] for chunk aq
                op = ps_pool.tile([P, 512], f32, name="op", tag="op")
                for h in range(GQ):
                    nc.tensor.matmul(
                        op[:],
                        y_sb[h][:, qb * P : (qb + 1) * P],
                        wo_sb[:, h, ct, :],
                        start=(h == 0),
                        stop=(h == GQ - 1),
                    )
                ot = ost_pool.tile([P, 512], bf16, name="ot", tag="ot")
                ev = evac_engines[o_count[0] % 2]
                if ev is nc.scalar:
                    nc.scalar.copy(ot[:], op[:])
                else:
                    nc.vector.tensor_copy(ot[:], op[:])
                oq = store_queues[o_count[0] % len(store_queues)]
                o_count[0] += 1
                oq.dma_start(
                    out[qb * P : (qb + 1) * P, ct * 512 : (ct + 1) * 512],
                    ot[:],
                )

            def make_units(aq):
                return [(aq, ct, qb) for ct in range(NCT)
                        for qb in range(4 * aq, 4 * aq + 4)]

            # deferred rope/V-transpose of the last projection chunk,
            # executed as PE filler inside attention chunk 0 (the o_proj
            # pool is idle there, so its PSUM banks host the rotate
            # matmuls / transposes).
            pq, praws, prawk, pvraw = pend_rope
            pq0 = pq * 512
            pcos = cos_sb[:, pq0 : pq0 + 512]
            psin = sin_sb[:, pq0 : pq0 + 512]

            def d_rot(raw, dst):
                def fn():
                    rp = o_ps.tile([P, 512], f32, name="rpd", tag="op")
                    nc.tensor.matmul(rp[:], rot_sb[:], raw[:], start=True,
                                     stop=True)
                    nc.vector.tensor_tensor(dst, raw[:], pcos, MULT)
                    t2 = nrm_pool.tile([P, 512], bf16, name="rt2d",
                                       tag="rt2d")
                    nc.vector.tensor_tensor(t2[:], rp[:], psin, MULT)
                    nc.vector.tensor_tensor(dst, dst, t2[:], ADD)
                return fn

            def d_vt(ks0, ks1):
                def fn():
                    for ks in (ks0, ks1):
                        tp = o_ps.tile([P, P], bf16, name="vtpd", tag="op")
                        nc.tensor.transpose(
                            tp[:], pvraw[:, ks * P : (ks + 1) * P], ident[:]
                        )
                        nc.vector.tensor_copy(v_sb[:, pq * 4 + ks, :], tp[:])
                return fn

            rope_fill = [d_rot(praws[h], qt_sb[h][:, pq0 : pq0 + 512])
                         for h in range(GQ)]
            rope_fill.append(d_rot(prawk, kt_sb[:, pq0 : pq0 + 512]))
            rope_fill.append(d_vt(0, 1))
            rope_fill.append(d_vt(2, 3))

            with (
                tc.tile_pool(name="s_ps", bufs=2, space="PSUM") as s_ps,
                tc.tile_pool(name="y_ps", bufs=1, space="PSUM") as y_ps,
                tc.tile_pool(name="rs_ps", bufs=1, space="PSUM") as rs_ps,
            ):
                for aq in range(NQC):
                    q0 = aq * 512
                    nks = 4 * (aq + 1)  # 128-wide k subtiles (incl 4 diagonal)
                    ng = nks // 2  # groups of 2 subtiles
                    units = make_units(aq - 1) if aq > 0 else []
                    slots = GQ * ng
                    credit = 0.0
                    ucount = len(units)

                    # narrowed (offset, width) per k-subtile: diagonal subtile
                    # m only covers q >= 128m within the 512-wide chunk.
                    def ow(ks):
                        m = ks - (nks - 4)
                        if m > 0:
                            return 128 * m, 512 - 128 * m
                        return 0, 512

                    for h in range(GQ):
                        qrow = qt_sb[h]
                        yp = y_ps.tile([P, 512], f32, name="yp", tag="yp")
                        rp_ = rs_ps.tile([P, 512], f32, name="rsp", tag="rsp")
                        sps = [None] * ng
                        # pair tiles awaiting their rowsum matmul:
                        # list of (tile, offA) in group order
                        pend_pairs = [None] * ng

                        def s_issue(g):
                            # the two subtiles are packed back to back in the
                            # sp tile ([0:w0], [w0:w0+w1]); w0 is always 256
                            # or 512 so neither matmul output crosses a PSUM
                            # bank.
                            sp = s_ps.tile([P, 1024], f32, name="sp", tag="sp")
                            off1 = 0
                            for ks in (2 * g, 2 * g + 1):
                                off, w = ow(ks)
                                nc.tensor.matmul(
                                    sp[:, off1 : off1 + w],
                                    kt_sb[:, ks * P : (ks + 1) * P],
                                    qrow[:, q0 + off : q0 + 512],
                                    start=True,
                                    stop=True,
                                )
                                off1 += w
                            sps[g] = sp

                        s_issue(0)
                        if ng > 1:
                            s_issue(1)
                        for g in range(ng):
                            if g + 2 < ng:
                                s_issue(g + 2)
                            # rowsum matmul for the PREVIOUS group's pair:
                            # emitted before this group's PV/direct matmuls
                            # so pair 0 (start=True) is always rp_'s first
                            # writer; its DVE add has had ~a full group to
                            # finish.
                            if g >= 1 and pend_pairs[g - 1] is not None:
                                pr, poff = pend_pairs[g - 1]
                                nc.tensor.matmul(
                                    rp_[:, poff:512],
                                    ones_sb[:],
                                    pr[:, poff:512],
                                    start=(g - 1 == 0),
                                    stop=False,
                                )
                                pend_pairs[g - 1] = None
                            # deferred last-chunk rope as PE filler (aq 0)
                            if rope_fill:
                                rope_fill.pop(0)()
                            # o_proj filler for the previous q-chunk
                            credit += ucount / slots
                            while credit >= 1.0 and units:
                                o_unit(*units.pop(0), o_ps, o_queues)
                                credit -= 1.0
                            sp = sps[g]
                            pt = pt_pool.tile([P, 1024], bf16, name="ptile",
                                              tag="pt")
                            subs = (2 * g, 2 * g + 1)
                            (offA, wA), (offB, wB) = ow(subs[0]), ow(subs[1])
                            wsum = wA + wB
                            nc.scalar.activation(
                                pt[:, 0:wsum], sp[:, 0:wsum], Exp, scale=SCALE
                            )
                            off1 = 0
                            for ks in subs:
                                w = ow(ks)[1]
                                if ks - (nks - 4) >= 0:
                                    # causal triangle on the first 128 cols
                                    # of the narrowed slice
                                    sl = pt[:, off1 : off1 + P]
                                    nc.vector.tensor_tensor(sl, sl, tri_sb[:],
                                                            MULT)
                                off1 += w
                            last_group = g == ng - 1
                            if not last_group:
                                # pair-reduce the two subtiles on DVE (bf16,
                                # one rounding per element); the rowsum
                                # matmul on the pair streams half the cols.
                                pair = pair_pool.tile([P, 512], bf16,
                                                      name="pair", tag="pair")
                                if offB > offA:
                                    # diagonal pair: [offA:offB] has only A
                                    nc.vector.tensor_copy(
                                        pair[:, offA:offB],
                                        pt[:, 0 : offB - offA],
                                    )
                                    nc.vector.tensor_tensor(
                                        pair[:, offB:512],
                                        pt[:, offB - offA : wA],
                                        pt[:, wA : wA + wB],
                                        ADD,
                                    )
                                else:
                                    nc.vector.tensor_tensor(
                                        pair[:, 0:512],
                                        pt[:, 0:512],
                                        pt[:, 512:1024],
                                        ADD,
                                    )
                                pend_pairs[g] = (pair, offA)
                            off1 = 0
                            for ks in subs:
                                off, w = ow(ks)
                                first, last = ks == 0, ks == nks - 1
                                prhs = pt[:, off1 : off1 + w]
                                off1 += w
                                nc.tensor.matmul(
                                    yp[:, off : off + w],
                                    v_sb[:, ks, :],
                                    prhs,
                                    start=first,
                                    stop=last,
                                )
                                if last_group:
                                    # final (diagonal) group: direct rowsum
                                    # matmuls (executed after pair 0's
                                    # start=True matmul) so nothing is
                                    # deferred across the head boundary.
                                    nc.tensor.matmul(
                                        rp_[:, off : off + w],
                                        ones_sb[:],
                                        prhs,
                                        start=False,
                                        stop=(ks == nks - 1),
                                    )
                        # 1/rowsum (~18 bits; rowsum >= 1 so no edge cases)
                        rinv = nrm_pool.tile([P, 512], f32, name="rinv",
                                             tag="rinv")
                        nc.vector.reciprocal_approx_fast(rinv[:], rp_[:])
                        nc.vector.tensor_tensor(
                            y_sb[h][:, q0 : q0 + 512], yp[:], rinv[:], MULT
                        )
                    # drain any leftover filler units of the previous chunk
                    for u in units:
                        o_unit(*u, o_ps, o_queues)
            # attention PSUM pools closed: 6 banks free. o_proj tail for the
            # last q-chunk runs from a 4-deep PSUM pool (pure matmul stream;
            # evacuation fully hidden), stores on sync/scalar only (gpsimd
            # issues nothing this late - its software-DGE drain is ~7.6us).
            tail_queues = (nc.sync, nc.scalar)
            with tc.tile_pool(name="o_tail_ps", bufs=4, space="PSUM") as o_tail:
                for u in make_units(NQC - 1):
                    o_unit(*u, o_tail, tail_queues)

    nc.compile()
    return nc


def _bf16(a):
    import ml_dtypes

    return np.ascontiguousarray(np.asarray(a, dtype=np.float32)).astype(
        ml_dtypes.bfloat16
    )


def make_in_maps(x, wq, wk, wv, wo, T=T_FULL):
    """Per-core input dicts for run_bass_kernel_spmd."""
    cosT, sinT = _rope_tables(T)
    tri = _tri128()
    onesm = np.ones((P, P), dtype=np.float32)
    rotm = _rot_lhsT()

    xts = [_bf16(x[b].T) for b in range(B)]
    cosT, sinT, tri, onesm, rotm = map(_bf16, (cosT, sinT, tri, onesm, rotm))
    in_maps = []
    for core in range(NCORES):
        b, g = core // 4, core % 4
        wkv = np.concatenate(
            (wk[:, D * g : D * (g + 1)], wv[:, D * g : D * (g + 1)]), axis=1
        )
        in_maps.append(
            {
                "xt": xts[b],
                "wq": _bf16(wq[:, 512 * g : 512 * (g + 1)]),
                "wkv": _bf16(wkv),
                "wo": _bf16(wo[512 * g : 512 * (g + 1), :]),
                "cosT": cosT,
                "sinT": sinT,
                "trim": tri,
                "onesm": onesm,
                "rotm": rotm,
            }
        )
    return in_maps


_NC_CACHE = {}


def _get_nc(T=T_FULL):
    if T not in _NC_CACHE:
        _NC_CACHE[T] = build_nc(T)
    return _NC_CACHE[T]


def run(inputs, trace=False):
    """Run on 8 NeuronCores. Returns (full_output, BassKernelResults)."""
    from concourse.bass_utils import run_bass_kernel_spmd

    x = np.asarray(inputs["x"], dtype=np.float32)
    in_maps = make_in_maps(
        x,
        np.asarray(inputs["wq"], dtype=np.float32),
        np.asarray(inputs["wk"], dtype=np.float32),
        np.asarray(inputs["wv"], dtype=np.float32),
        np.asarray(inputs["wo"], dtype=np.float32),
    )
    nc = _get_nc()
    res = run_bass_kernel_spmd(nc, in_maps, list(range(NCORES)), trace=trace)
    outs = res.results
    full = np.zeros((B, T_FULL, C_DIM), dtype=np.float32)
    for core in range(NCORES):
        full[core // 4] += np.asarray(outs[core]["out"], dtype=np.float32)
    return full, res


def kernel(**inputs):
    full, _ = run(inputs, trace=False)
    return full
